# revision 32
# baseline (speedup 1.0000x reference)
"""Sliding-window (radius-8, K=17) single-head attention along W.

Full problem: feature/position [2, 128, 64, 256] f32; 1x1 convs Wq/Wk (+bias)
produce q/k; scores over a 17-wide window along W; softmax (zero-padded
windows contribute exp(0)=1 to the denominator); output is the attn-weighted
sum of windows of x = feature + position.

Sharding: data-parallel over (B, H) — the 128 (b, h) rows are independent;
each of the 8 cores gets 16 rows, two per iteration.

Per row (x_row = [C=128, W=256], x held in bf16; measured end-to-end rel err
~6e-3 vs the fp32 reference):
  q = (Wq/sqrt(C)) x + bq/sqrt(C);  k = Wk x + bk     (bf16 PE matmuls, fp32
      PSUM accumulate, bias added during the f32r eviction)
  S^T[w', w] = k^T q   computed TRANSPOSED (keys on partitions) in f32r so
      exp sees near-fp32 scores and exp(S^T) lands in SBUF in the layout the
      den/out matmuls need.
  Band structure: key chunk 1 (keys 0..127) only reaches queries 0..135;
  chunk 2 (keys 128..255) only queries 120..255. exp / mask / den / out all
  operate on those 136-wide strips only (scores are computed full-width —
  f32r matmuls need >=256 moving cols for 1 cyc/row — but never read
  outside the strips). Masking is multiplicative 0/1 on exp(S) post-exp.
  den[w] (broadcast across partitions) = ones128.T @ att strips, PSUM-
      initialized by ones128.T @ (oobcount/128) for the zero-padded
      out-of-range taps (exp(0)=1 each); out = (x^T.T @ att) * recip(den).
  x^T chunks from bf16 PE transposes of x.

Schedule: two-stage software pipeline. Stage A(i) = x-add (gpsimd, bf16
out), q/k matmuls, x^T transposes (issued before the score matmuls so the
PE has dependency-free work while the bias evictions run on scalar), score
matmuls, one merged strip-exp (scalar), one merged strip-mask (vector),
x^T eviction (scalar). Stage B(i) = den matmuls (3), reciprocal (vector),
out matmuls (6), final normalize (vector), batched output DMA. Issue order
A(0), A(1), B(0), A(2), B(1), ... Input DMAs: rows 0-1 first, then one
bf16 constant blob, rows 2-3, rows 4-15, so compute starts early and the
sync queue issues few DMA instructions.
"""

import numpy as np
from contextlib import ExitStack

import concourse.bacc as bacc
import concourse.mybir as mybir
import concourse.tile as tile
from concourse.ap import AP
from concourse.bass_utils import run_bass_kernel_spmd

# Enable the walrus ldw-opt pass (dedupes redundant LDWEIGHTS between
# consecutive matmuls sharing a stationary operand). Off by default in
# bass_utils; output is bit-identical for this kernel and ~1.3us faster.
import concourse.bass_utils as _bu

if not getattr(_bu, "_ldwopt_patched", False):
    _orig_walrus_args = _bu.get_walrus_args

    def _walrus_args_ldwopt(arch, tmpdir, *, dve_root=None):
        args = _orig_walrus_args(arch, tmpdir, dve_root=dve_root)
        return [
            a.replace("--enable-ldw-opt=false", "--enable-ldw-opt=true")
            for a in args
        ]

    _bu.get_walrus_args = _walrus_args_ldwopt
    _bu._ldwopt_patched = True

B, C, H, W = 2, 128, 64, 256
R = 8
NCORES = 8
ROWS = B * H // NCORES        # 16 (b, h) rows per core
CORES_PER_B = NCORES // B     # 4
F32 = mybir.dt.float32
F32R = mybir.dt.float32r
F16 = mybir.dt.float16
BF = mybir.dt.bfloat16
EXP = mybir.ActivationFunctionType.Exp
COPY = mybir.ActivationFunctionType.Copy
SW = 136                      # strip width: chunk1 queries [0:136), chunk2 [120:256)
A1 = 2 * W - SW               # chunk2 strip start within a row's 512 att cols (376)
CB = 1568                     # bf16 const blob cols: wq|wk|ident|ones|mask(544)|oob(512)


def apn(t, dims, off=0):
    v = t[:]
    return AP(v.tensor, v.offset + off, list(v.ap[:1]) + list(dims))


def build_nc():
    nc = bacc.Bacc(trn_type="TRN2")
    f_ext = nc.dram_tensor("feature", [C, ROWS, W], F16, kind="ExternalInput")
    p_ext = nc.dram_tensor("position", [C, ROWS, W], F16, kind="ExternalInput")
    cb_ext = nc.dram_tensor("constb", [C, CB], BF, kind="ExternalInput")
    cf_ext = nc.dram_tensor("constf", [C, 2], F32, kind="ExternalInput")
    out_ext = nc.dram_tensor("out", [C, ROWS, W], F16, kind="ExternalOutput")

    with tile.TileContext(nc) as tc, ExitStack() as ctx:
        const = ctx.enter_context(tc.tile_pool(name="const", bufs=1))
        inp = ctx.enter_context(tc.tile_pool(name="inp", bufs=5))

        blocks = {}   # iter -> (ft, pt, j): input tile pair + row offset

        def load_rows(r0, nrows, iters):
            # f via the sync HWDGE, p via the scalar HWDGE: two issue queues
            # in parallel so early blocks land sooner
            ft = inp.tile([C, nrows, W], F16, tag="ft")
            nc.sync.dma_start(ft[:], f_ext[:, r0 : r0 + nrows, :])
            pt = inp.tile([C, nrows, W], F16, tag="pt")
            nc.sync.dma_start(pt[:], p_ext[:, r0 : r0 + nrows, :])
            for n, it in enumerate(iters):
                blocks[it] = (ft, pt, 2 * n)

        # rows 0-1 and the constant blob land first so compute starts early
        load_rows(0, 2, [0])
        cb = const.tile([C, CB], BF, tag="cb")
        nc.sync.dma_start(cb[:], cb_ext[:])
        cf = const.tile([C, 2], F32, tag="cf")
        nc.sync.dma_start(cf[:], cf_ext[:])
        load_rows(2, 2, [1])
        load_rows(4, 4, [2, 3])
        load_rows(8, 8, [4, 5, 6, 7])

        wq_t = cb[:, 0:128]
        wk_t = cb[:, 128:256]
        ident = cb[:, 256:384]
        ones_t = cb[:, 384:512]
        mask_f = cb[:, 512:1056]
        oob_v = cb[:, 1056:1568]
        bq_t = cf[:, 0:1]
        bk_t = cf[:, 1:2]

        # touch Exp once so the ACT table loads during the input-DMA ramp
        warm = const.tile([C, 1], F32, tag="warm")
        nc.scalar.activation(warm[:], bq_t, EXP)

        xp = ctx.enter_context(tc.tile_pool(name="x", bufs=3))
        qkp = ctx.enter_context(tc.tile_pool(name="qk", bufs=3))
        attp = ctx.enter_context(tc.tile_pool(name="att", bufs=3))
        sbT = ctx.enter_context(tc.tile_pool(name="sbT", bufs=3))
        rdp = ctx.enter_context(tc.tile_pool(name="rd", bufs=2))
        osp = ctx.enter_context(tc.tile_pool(name="os", bufs=2))
        psq = ctx.enter_context(tc.tile_pool(name="psq", bufs=1, space="PSUM"))
        psk = ctx.enter_context(tc.tile_pool(name="psk", bufs=1, space="PSUM"))
        pss = ctx.enter_context(tc.tile_pool(name="pss", bufs=1, space="PSUM"))
        psdo = ctx.enter_context(tc.tile_pool(name="psdo", bufs=2, space="PSUM"))

        NIT = ROWS // 2
        st = {}
        osb = {}
        qk4 = {}
        rdn = {}

        def stageA(it):
            si, h = divmod(it, 2)
            if h == 0:
                x4 = xp.tile([C, 4 * W], BF, tag="x4")
                q_ps = psq.tile([C, 2, 2 * W], F32, tag="q")
                k_ps = psk.tile([C, 2, 2 * W], F32, tag="k")
                q_sb4 = qkp.tile([C, 2, 2 * W], F32R, tag="q")
                k_sb4 = qkp.tile([C, 2, 2 * W], F32R, tag="k")
                qk4[si] = (x4, q_ps, k_ps, q_sb4, k_sb4)
            x4, q_ps, k_ps, q_sb4, k_sb4 = qk4[si]

            if si <= 1:
                # ramp: 2-row granularity so the first scores start as soon
                # as the early input blocks land
                ft, pt, j = blocks[it]
                xv = AP(x4[:].tensor, x4[:].offset + h * 2 * W, [(4 * W, C), (W, 2), (1, W)])
                nc.gpsimd.tensor_add(xv, ft[:, j : j + 2, :], pt[:, j : j + 2, :])
                nc.tensor.matmul(
                    q_ps[:, h], wq_t, x4[:, h * 2 * W : (h + 1) * 2 * W],
                    start=True, stop=True,
                )
                nc.tensor.matmul(
                    k_ps[:, h], wk_t, x4[:, h * 2 * W : (h + 1) * 2 * W],
                    start=True, stop=True,
                )
                nc.scalar.add(q_sb4[:, h], q_ps[:, h], bq_t)
                nc.scalar.add(k_sb4[:, h], k_ps[:, h], bk_t)
            elif h == 0:
                # steady state: four rows per gpsimd add / weight load
                ft, pt, j = blocks[it]
                nc.gpsimd.tensor_add(
                    apn(x4, [(W, 4), (1, W)]),
                    ft[:, j : j + 4, :],
                    pt[:, j : j + 4, :],
                )
                nc.tensor.matmul(q_ps[:, 0], wq_t, x4[:, 0 : 2 * W], start=True, stop=True)
                nc.tensor.matmul(q_ps[:, 1], wq_t, x4[:, 2 * W : 4 * W], start=True, stop=True)
                nc.tensor.matmul(k_ps[:, 0], wk_t, x4[:, 0 : 2 * W], start=True, stop=True)
                nc.tensor.matmul(k_ps[:, 1], wk_t, x4[:, 2 * W : 4 * W], start=True, stop=True)
                nc.scalar.add(q_sb4[:], q_ps[:], bq_t)
                nc.scalar.add(k_sb4[:], k_ps[:], bk_t)
            q_sb = q_sb4[:, h]
            k_sb = k_sb4[:, h]
            x2v = x4[:, h * 2 * W : (h + 1) * 2 * W]

            # x^T chunks via PE transposes (bf16 data, bf16 identity)
            xt_ps = psdo.tile([C, 4, 128], BF, tag="do")
            for b4 in range(4):
                nc.tensor.transpose(
                    xt_ps[:, b4, :], x2v[:, b4 * 128 : (b4 + 1) * 128], ident
                )
            xT = sbT.tile([C, 4, 128], BF, tag="xT")
            nc.scalar.activation(xT[:, 0:2, :], xt_ps[:, 0:2, :], COPY)
            nc.vector.tensor_copy(xT[:, 2:4, :], xt_ps[:, 2:4, :])

            # scores per row, transposed: [C, 2, 512] across 2 PSUM banks
            s_ps = pss.tile([C, 2, 2 * W], F32, tag="s")
            for rr in range(2):
                q0 = rr * W
                nc.tensor.matmul(
                    s_ps[:, rr, 0:W],
                    k_sb[:, q0 : q0 + 128],
                    q_sb[:, q0 : q0 + W],
                    start=True, stop=True,
                )
                nc.tensor.matmul(
                    s_ps[:, rr, W : 2 * W],
                    k_sb[:, q0 + 128 : q0 + W],
                    q_sb[:, q0 : q0 + W],
                    start=True, stop=True,
                )
            st[it] = (s_ps, xT)

        def stageA2(it):
            s_ps, xT = st[it]
            # exp of the valid strips into a strips-only [C, 2, 2, SW] tile
            # (contiguous writes; mask and den/out reads all contiguous)
            att = attp.tile([C, 2, 2, SW], BF)
            nc.scalar.activation(
                att[:], apn(s_ps, [(2 * W, 2), (A1, 2), (1, SW)]), EXP
            )
            # multiplicative 0/1 band mask, both rows in one flat op
            nc.vector.tensor_mul(
                apn(att, [(1, 4 * SW)]), apn(att, [(1, 4 * SW)]), mask_f
            )
            st[it] = (att, xT)

        def den_recip(it):
            att, xT = st[it]
            # denominators, broadcast across partitions by the ones matmul;
            # PSUM-initialized with the oob counts (pre-divided by 128).
            den_ps = psdo.tile([C, 2 * W], F32, tag="do")
            nc.tensor.matmul(den_ps[:], ones_t, oob_v, start=True, stop=False)
            nc.tensor.matmul(
                apn(den_ps, [(W, 2), (1, SW)]),
                ones_t,
                att[:, :, 0, :],
                start=False, stop=False,
            )
            nc.tensor.matmul(
                apn(den_ps, [(W, 2), (1, SW)], off=W - SW),
                ones_t,
                att[:, :, 1, :],
                start=False, stop=True,
            )
            rden = rdp.tile([C, 2 * W], F32)
            nc.vector.reciprocal_approx_fast(out=rden[:], in_=den_ps[:])
            rdn[it] = rden

        def stageB(it):
            r = 2 * it
            att, xT = st.pop(it)
            rden = rdn.pop(it)

            o_ps = psdo.tile([C, 2 * W], F32, tag="do")
            for rr in range(2):
                o0 = rr * W
                nc.tensor.matmul(
                    o_ps[:, o0 : o0 + SW],
                    xT[:, 2 * rr, :],
                    att[:, rr, 0, :],
                    start=True, stop=False,
                )
                nc.tensor.matmul(
                    o_ps[:, o0 + W - SW : o0 + SW],
                    xT[:, 2 * rr + 1, :],
                    att[:, rr, 1, 0:16],
                    start=False, stop=True,
                )
                nc.tensor.matmul(
                    o_ps[:, o0 + SW : o0 + W],
                    xT[:, 2 * rr + 1, :],
                    att[:, rr, 1, 16:SW],
                    start=True, stop=True,
                )
            o_sb = osp.tile([C, 2, W], F16, tag="osb")
            nc.vector.tensor_mul(o_sb[:], o_ps[:], rden[:])
            nc.sync.dma_start(out_ext[:, r : r + 2, :], o_sb[:])

        stageA(0)
        stageA2(0)
        for it in range(1, NIT):
            stageA(it)
            den_recip(it - 1)
            stageA2(it)
            stageB(it - 1)
        den_recip(NIT - 1)
        stageB(NIT - 1)

    nc.compile()
    return nc


def host_consts(Wq, bq, Wk, bk):
    import ml_dtypes

    sc = 1.0 / np.sqrt(np.float32(C))
    wqt = np.ascontiguousarray(Wq.astype(np.float32).T * sc)
    wkt = np.ascontiguousarray(Wk.astype(np.float32).T)
    ident = np.eye(C, dtype=np.float32)
    ones = np.ones((C, C), dtype=np.float32)

    # 0/1 band masks on the two valid strips (same for both rows):
    # chunk1: key p vs query w in [0, SW);  chunk2: key 128+p vs query 120+j
    maskc = np.zeros((C, 2, SW), dtype=np.float32)
    for p in range(C):
        for w in range(SW):
            if abs(p - w) <= R:
                maskc[p, 0, w] = 1.0
            if abs((128 + p) - (W - SW + w)) <= R:
                maskc[p, 1, w] = 1.0
    maskc = np.broadcast_to(maskc[:, None], (C, 2, 2, SW)).reshape(C, 4 * SW)

    # oob count per query w (pre-divided by 128: the ones-matmul sums over
    # 128 partitions), same row repeated on all partitions, two rows
    wgrid = np.arange(W)
    oob_row = (np.maximum(0, R - wgrid) + np.maximum(0, wgrid - (W - 1 - R))) / 128.0
    oob_bc = np.tile(oob_row.astype(np.float32), (C, 2))

    constb = np.concatenate(
        [wqt, wkt, ident, ones, maskc, oob_bc], axis=1
    ).astype(ml_dtypes.bfloat16)
    assert constb.shape == (C, CB), constb.shape
    constf = np.stack(
        [bq.astype(np.float32) * sc, bk.astype(np.float32)], axis=1
    ).reshape(C, 2)
    return np.ascontiguousarray(constb), np.ascontiguousarray(constf)


def core_inputs(feature, position, Wq, bq, Wk, bk):
    constb, constf = host_consts(Wq, bq, Wk, bk)
    in_maps = []
    for i in range(NCORES):
        b = i // CORES_PER_B
        h0 = (i % CORES_PER_B) * ROWS
        in_maps.append(
            {
                "feature": np.ascontiguousarray(
                    feature[b, :, h0 : h0 + ROWS, :], dtype=np.float16
                ),
                "position": np.ascontiguousarray(
                    position[b, :, h0 : h0 + ROWS, :], dtype=np.float16
                ),
                "constb": constb,
                "constf": constf,
            }
        )
    return in_maps


def kernel(feature, position, Wq, bq, Wk, bk):
    feature = np.asarray(feature, dtype=np.float32)
    position = np.asarray(position, dtype=np.float32)
    Wq = np.asarray(Wq, dtype=np.float32)
    bq = np.asarray(bq, dtype=np.float32)
    Wk = np.asarray(Wk, dtype=np.float32)
    bk = np.asarray(bk, dtype=np.float32)
    in_maps = core_inputs(feature, position, Wq, bq, Wk, bk)
    nc = build_nc()
    res = run_bass_kernel_spmd(nc, in_maps, list(range(NCORES)))
    out = np.empty((B, C, H, W), dtype=np.float32)
    for i in range(NCORES):
        b = i // CORES_PER_B
        h0 = (i % CORES_PER_B) * ROWS
        out[b, :, h0 : h0 + ROWS, :] = res.results[i]["out"].astype(np.float32)
    return out
